# revision 1
# baseline (speedup 1.0000x reference)
"""FFF (fast feedforward / MoE tree-routing) Trainium2 kernel, v3.

Two launches, 8 cores SPMD, bf16 with host margin-fixup:

  Launch 1 -- dense routing levels 0..7, data-parallel (1024 samples/core).
    All 255 shallow-node scores per sample via bf16 matmuls (PE p-state
    warmed by dummy matmuls so the stream runs at full clock); per-level
    select via iota/is_equal masking on VectorE in two half-chains.
    Emits the standing level-8 node plus min squared decision margin.

  Host -- all-to-all dispatch keyed on the level-8 node: each core owns 32
    of the 256 depth-8 subtrees (= 8-expert groups).  Per core, groups are
    rank-ordered by size, so one SPMD NEFF with shared per-rank capacities
    fits all cores tightly; first-fit-decreasing bin packing into 128-slot
    blocks, with the weight stream reordered to block order so block
    completion is monotone.  Weights are repacked to bf16.

  Launch 2 -- expert-parallel fused MLP + final 3 routing levels, fully
    overlapped with a single gapless weight stream (W1 parts batched 4
    ranks per DMA up front; W2 parts deferred per block to its finalize so
    the launch tail overlaps the last transfers; output stores issued from
    the Activation/SP queues so loads never stall).  Per group: a
    [768->128] stacked-W1 bf16 matmul chain computes all 8 experts' hidden
    lanes into a per-rank PSUM bank (accumulation chains must be
    contiguous per bank), relu with the group's per-lane bias applied via
    the activation bias port; a parallel [768->8] matmul scores the
    group's depth-3 subtree; batched VectorE ops turn scores into expert
    one-hots; a positional mask matmul (4-rank transposed clusters)
    produces each block's lane mask which multiplies the hidden tile.  The
    second layer runs output-major so PSUM evacuation amortizes per block.
    Min squared subtree margins are emitted per slot.

  Host -- scatter slots back to sample order; recompute the few samples
    whose routing margin at either stage was below threshold (decisions
    there are within device-arithmetic error of the fp32 reference).
"""

import contextlib
import numpy as np

import concourse.bacc as bacc
import concourse.mybir as mybir
import concourse.tile as tile
from concourse.bass import ts
from concourse.mybir import ActivationFunctionType, AluOpType, AxisListType
from concourse.bass_utils import run_bass_kernel_spmd

# problem shapes (hardcoded per contract)
DEPTH = 11
IN_W = 768
LEAF_W = 16
OUT_W = 768
N_NODES = 2047
N_LEAVES = 2048
BATCH = 8192
N_CORES = 8

B_CORE = BATCH // N_CORES            # 1024
KC = IN_W // 128                     # 6 contraction chunks
DENSE_LEVELS = 8                     # levels 0..7 dense -> 256 subtrees
N_GROUPS = 2 ** DENSE_LEVELS         # 256 8-expert groups
GROUPS_PER_CORE = N_GROUPS // N_CORES  # 32

BIG = 16384.0                        # anti-mask magnitude (bf16-exact)
MARGIN1 = 2.5e-2                     # bf16 dense-score abs-error threshold
MARGIN2 = 2.5e-2                     # bf16 subtree-score abs-error threshold

F32 = mybir.dt.float32
F32R = mybir.dt.float32r
BF16 = mybir.dt.bfloat16
I32 = mybir.dt.int32

F_COPY = ActivationFunctionType.Copy
F_RELU = ActivationFunctionType.Relu

LAST_CAPS = [128] * GROUPS_PER_CORE  # capacities used by last kernel() call
DEBUG_L2 = False                     # adds intermediate dumps to launch 2


# ---------------------------------------------------------------- launch 1
def _build_l1_nc():
    nc = bacc.Bacc("TRN2", target_bir_lowering=False, debug=False,
                   num_devices=N_CORES)
    xT = nc.dram_tensor("xT", [IN_W, B_CORE], BF16, kind="ExternalInput").ap()
    wd = nc.dram_tensor("wd", [IN_W, 256], BF16, kind="ExternalInput").ap()
    wdb = nc.dram_tensor("wdb", [1, 256], BF16, kind="ExternalInput").ap()
    gout = nc.dram_tensor("gout", [128, 16], I32, kind="ExternalOutput").ap()

    with tile.TileContext(nc) as tc, contextlib.ExitStack() as ctx:
        pool = ctx.enter_context(tc.tile_pool(name="sbuf", bufs=1))
        psum = ctx.enter_context(tc.tile_pool(name="psum", bufs=1,
                                              space="PSUM"))

        wd_sb = pool.tile([128, KC, 256], BF16)
        wdb_sb = pool.tile([1, 256], BF16)
        ones_sb = pool.tile([1, B_CORE], BF16)
        xt_sb = pool.tile([128, KC, B_CORE], BF16)
        nc.vector.memset(ones_sb[:], 1.0)
        xt_r = xT.rearrange("(k p) s -> p k s", p=128)
        nc.sync.dma_start(out=xt_sb[:, :, 0:256], in_=xt_r[:, :, 0:256])
        nc.sync.dma_start(out=wd_sb[:],
                          in_=wd.rearrange("(k p) n -> p k n", p=128))
        nc.sync.dma_start(out=wdb_sb[:], in_=wdb)
        for q in range(1, 4):
            nc.sync.dma_start(out=xt_sb[:, :, 256 * q:256 * q + 256],
                              in_=xt_r[:, :, 256 * q:256 * q + 256])

        # tensor-engine p-state warmup: wide back-to-back matmuls with no
        # DMA deps keep PE continuously busy until the inputs land so the
        # real score matmuls run at full clock (the cost model's ramp needs
        # ~3us of uninterrupted engine busy)
        wm = pool.tile([128, 256], BF16)
        nc.vector.memset(wm[:], 1.0)
        wp = psum.tile([128, 256], F32, tag="warm")
        for _ in range(10):
            nc.tensor.matmul(wp[:], lhsT=wm[:, 0:128], rhs=wm[:, 0:256],
                             start=True, stop=True)

        # dense scores: 8 c-tiles of 128 samples x 256 node columns
        s_sb = pool.tile([128, 8, 256], BF16)
        for c in range(8):
            ps = psum.tile([128, 256], F32, tag="ps", name=f"ps{c}", bufs=4)
            for k in range(KC):
                nc.tensor.matmul(ps[:], lhsT=xt_sb[:, k, ts(c, 128)],
                                 rhs=wd_sb[:, k, :], start=(k == 0),
                                 stop=False)
            nc.tensor.matmul(ps[:], lhsT=ones_sb[:, ts(c, 128)],
                             rhs=wdb_sb[:], start=False, stop=True)
            nc.scalar.activation(out=s_sb[:, c, :], in_=ps[:], func=F_COPY)

        # per-level tree walk: two half-chains, emission interleaved so the
        # second half's ops fill the first chain's dependency gaps on DVE
        iota_i = pool.tile([128, 4, 256], I32)
        iota_f = pool.tile([128, 4, 256], BF16)
        nc.gpsimd.iota(iota_i[:], pattern=[[0, 4], [1, 256]], base=0,
                       channel_multiplier=0)
        nc.vector.tensor_copy(out=iota_f[:], in_=iota_i[:])

        gi = pool.tile([128, 16], I32)
        mo = pool.tile([128, 8], F32)
        hs = {}
        for h in range(2):
            hs[h] = dict(
                csl=slice(4 * h, 4 * h + 4),
                r=pool.tile([128, 4], BF16, tag=f"r{h}", name=f"r{h}"),
                ch=pool.tile([128, 4], BF16, tag=f"ch{h}", name=f"ch{h}"),
                sel=pool.tile([128, 4], F32, tag=f"sel{h}", name=f"sel{h}"),
                selb=pool.tile([128, 4, 8], F32, tag=f"sb{h}", name=f"sb{h}"),
                mask=pool.tile([128, 4, 128], BF16, tag=f"mk{h}",
                               name=f"mk{h}"),
                prod=pool.tile([128, 4, 128], BF16, tag=f"pr{h}",
                               name=f"pr{h}"),
            )

        def walk_start(h):
            v = hs[h]
            eng = nc.vector
            nc.scalar.activation(out=v["selb"][:, :, 0],
                                 in_=s_sb[:, v["csl"], 0], func=F_COPY)
            eng.tensor_scalar(out=v["r"][:], in0=s_sb[:, v["csl"], 0],
                              scalar1=0.0, scalar2=None,
                              op0=AluOpType.is_ge)

        def walk_level_ops(h, lvl):
            """Yield one level's ops as thunks; half 1 runs on gpsimd."""
            v = hs[h]
            eng = nc.vector
            n = 2 ** lvl
            off = n - 1
            yield lambda: eng.tensor_tensor(
                out=v["mask"][:, :, :n], in0=iota_f[:, :, :n],
                in1=v["r"][:, :, None].to_broadcast([128, 4, n]),
                op=AluOpType.is_equal)
            yield lambda: eng.tensor_tensor(
                out=v["prod"][:, :, :n], in0=v["mask"][:, :, :n],
                in1=s_sb[:, v["csl"], off:off + n], op=AluOpType.mult)
            yield lambda: eng.tensor_reduce(
                out=v["sel"][:], in_=v["prod"][:, :, :n],
                axis=AxisListType.X, op=AluOpType.add)

            def _tail():
                nc.scalar.activation(out=v["selb"][:, :, lvl],
                                     in_=v["sel"][:], func=F_COPY)
                eng.tensor_scalar(out=v["ch"][:], in0=v["sel"][:],
                                  scalar1=0.0, scalar2=None,
                                  op0=AluOpType.is_ge)
            yield _tail
            yield lambda: eng.scalar_tensor_tensor(
                out=v["r"][:], in0=v["r"][:], scalar=2.0, in1=v["ch"][:],
                op0=AluOpType.mult, op1=AluOpType.add)

        def walk_end_ops(h):
            v = hs[h]
            eng = nc.vector
            csl = v["csl"]
            ab = pool.tile([128, 4, 8], F32, tag=f"ab{h}", name=f"abt{h}")
            yield lambda: eng.tensor_tensor(
                out=ab[:], in0=v["selb"][:], in1=v["selb"][:],
                op=AluOpType.mult)
            yield lambda: eng.tensor_reduce(
                out=mo[:, csl], in_=ab[:], axis=AxisListType.X,
                op=AluOpType.min)
            yield lambda: eng.tensor_copy(out=gi[:, csl], in_=v["r"][:])
            yield lambda: eng.tensor_copy(
                out=gi[:, 8:16][:, csl], in_=mo[:, csl].bitcast(I32))

        def chain_ops(h, lvls, end=False):
            for lvl in lvls:
                yield from walk_level_ops(h, lvl)
            if end:
                yield from walk_end_ops(h)

        walk_start(0)
        for op in chain_ops(0, range(1, 6)):
            op()
        walk_start(1)
        a_rest = list(chain_ops(0, (6, 7), end=True))
        b_all = list(chain_ops(1, range(1, 8), end=True))
        ia, ib = 0, 0
        while ia < len(a_rest) or ib < len(b_all):
            if ia < len(a_rest):
                a_rest[ia]()
                ia += 1
            if ib < len(b_all):
                b_all[ib]()
                ib += 1

        nc.scalar.dma_start(out=gout, in_=gi[:])

    nc.compile()
    return nc


# ---------------------------------------------------------------- launch 2
def _plan_blocks(caps):
    """First-fit-decreasing pack of ranked capacities into 128-slot blocks,
    then relabel ranks in block order so the weight stream and block
    completion stay monotone.

    Returns (norder, pcaps, plan, T): norder[i] = original rank at stream
    position i; pcaps[i] its capacity; plan[i] = (block, offset_in_block).
    """
    bins = []
    for r, cap in enumerate(caps):
        if cap == 0:
            continue
        assert cap <= 128
        for b in bins:
            if b[0] + cap <= 128:
                b[1].append((r, b[0]))
                b[0] += cap
                break
        else:
            bins.append([cap, [(r, 0)]])
    norder, pcaps, plan = [], [], {}
    for t, b in enumerate(bins):
        for r, off in b[1]:
            plan[len(norder)] = (t, off)
            norder.append(r)
            pcaps.append(caps[r])
    return norder, pcaps, plan, len(bins)


def _build_l2_nc(caps):
    norder, pcaps, plan, T = _plan_blocks(caps)
    SLOTS = 128 * T
    W1_W = KC * 128 + 48             # w1 stack | subtree w8
    W2_W = OUT_W
    NG = GROUPS_PER_CORE
    blocks = [[] for _ in range(T)]
    for r, (t, off) in plan.items():
        blocks[t].append((r, off, pcaps[r]))
    R = len(norder)
    block_last = {t: max(r for r, _, _ in bl)
                  for t, bl in enumerate(blocks) if bl}

    nc = bacc.Bacc("TRN2", target_bir_lowering=False, debug=False,
                   num_devices=N_CORES)
    xgT = nc.dram_tensor("xgT", [IN_W, SLOTS], BF16, kind="ExternalInput").ap()
    wsl1 = nc.dram_tensor("wsl1", [NG, 128, W1_W], BF16,
                          kind="ExternalInput").ap()
    wsl2 = nc.dram_tensor("wsl2", [NG, 128, W2_W], BF16,
                          kind="ExternalInput").ap()
    b8r = nc.dram_tensor("b8r", [1, NG * 8], BF16, kind="ExternalInput").ap()
    b1c = nc.dram_tensor("b1c", [128, NG], BF16, kind="ExternalInput").ap()
    m32b = nc.dram_tensor("m32b", [32, 4 * 128], BF16,
                          kind="ExternalInput").ap()
    ident = nc.dram_tensor("ident", [128, 128], BF16,
                           kind="ExternalInput").ap()
    outT = nc.dram_tensor("outT", [OUT_W, SLOTS], BF16,
                          kind="ExternalOutput").ap()
    mg2 = nc.dram_tensor("mg2", [128, NG], BF16, kind="ExternalOutput").ap()
    if DEBUG_L2:
        dstg = nc.dram_tensor("dstg", [128, NG * 8], F32,
                              kind="ExternalOutput").ap()
        dao = nc.dram_tensor("dao", [128, NG * 8], F32,
                             kind="ExternalOutput").ap()
        dhf = nc.dram_tensor("dhf", [128, T * 128], F32,
                             kind="ExternalOutput").ap()

    with tile.TileContext(nc) as tc, contextlib.ExitStack() as ctx:
        # psum pools created big-to-small so matmul targets stay bank-aligned
        pop = ctx.enter_context(tc.tile_pool(name="pop", bufs=1, space="PSUM"))
        p1p = ctx.enter_context(tc.tile_pool(name="p1p", bufs=3, space="PSUM"))
        mop = ctx.enter_context(tc.tile_pool(name="mop", bufs=1, space="PSUM"))
        scp = ctx.enter_context(tc.tile_pool(name="scp", bufs=1, space="PSUM"))
        trp = ctx.enter_context(tc.tile_pool(name="trp", bufs=1, space="PSUM"))
        pool = ctx.enter_context(tc.tile_pool(name="sbuf", bufs=1))
        wpool = ctx.enter_context(tc.tile_pool(name="w", bufs=8))
        w2pool = ctx.enter_context(tc.tile_pool(name="w2", bufs=3))
        hpool = ctx.enter_context(tc.tile_pool(name="h", bufs=12))
        apool = ctx.enter_context(tc.tile_pool(name="a", bufs=10))
        opool = ctx.enter_context(tc.tile_pool(name="o", bufs=3))

        xt_sb = pool.tile([128, KC, SLOTS], BF16)
        xg_r = xgT.rearrange("(k p) s -> p k s", p=128)
        S0 = min(384, SLOTS)
        nc.sync.dma_start(out=xt_sb[:, :, 0:S0], in_=xg_r[:, :, 0:S0])
        ones_sb = pool.tile([1, SLOTS], BF16)
        nc.vector.memset(ones_sb[:], 1.0)
        b8_sb = pool.tile([1, NG * 8], BF16)
        nc.scalar.dma_start(out=b8_sb[:], in_=b8r)
        b1_sb = pool.tile([128, NG], BF16)
        nc.scalar.dma_start(out=b1_sb[:], in_=b1c)
        m32_sb = pool.tile([32, 4, 128], BF16)
        nc.scalar.dma_start(out=m32_sb[:],
                            in_=m32b.rearrange("q (c l) -> q c l", c=4))
        id_sb = pool.tile([128, 128], BF16)
        nc.scalar.dma_start(out=id_sb[:], in_=ident)

        iota_i = pool.tile([128, NG, 8], I32)
        i8 = pool.tile([128, NG, 8], BF16)
        nc.gpsimd.iota(iota_i[:], pattern=[[0, NG], [1, 8]], base=0,
                       channel_multiplier=0)
        nc.vector.tensor_copy(out=i8[:], in_=iota_i[:])

        stg = pool.tile([128, NG, 8], BF16)   # per-rank staged subtree scores
        ao = pool.tile([128, NG, 8], BF16)    # per-rank anti-one-hot
        mgt = pool.tile([128, NG], BF16)      # per-rank margins

        w_tiles, hf_tiles, at_tiles = {}, {}, {}
        osb_tiles, osb_cnt = {}, {}
        sc_rot = 0

        bounds = sorted(set(list(range(0, R, 24)) + [max(R - 8, 0), max(R - 4, 0), R]))
        for lo, hi in zip(bounds[:-1], bounds[1:]):
            # --- stream weights; first-layer + subtree-score matmuls
            for g0 in range(lo, hi, 4):
                g1 = min(g0 + 4, hi)
                grp = [r for r in range(g0, g1) if r in plan]
                if not grp:
                    continue
                wg = wpool.tile([128, len(grp), W1_W], BF16, tag="w",
                                name=f"wg{g0}")
                nc.sync.dma_start(
                    out=wg[:],
                    in_=wsl1[grp[0]:grp[-1] + 1].rearrange("g p w -> p g w"))
                for i, r in enumerate(grp):
                    w_tiles[r] = wg[:, i, :]
                if g0 == lo == 0 and SLOTS > S0:
                    nc.sync.dma_start(out=xt_sb[:, :, S0:SLOTS],
                                      in_=xg_r[:, :, S0:SLOTS])
            sc = scp.tile([128, KC, 8], F32, tag="sc", name=f"sc_{lo}")
            for r in range(lo, hi):
                if r not in plan:
                    continue
                t, ob = plan[r]
                cap = pcaps[r]
                o = 128 * t + ob
                if t not in hf_tiles:
                    hf_tiles[t] = hpool.tile([128, 128], BF16, tag="hf",
                                             name=f"hf_{t}")
                hf = hf_tiles[t]
                w = w_tiles[r]
                j = sc_rot % KC
                sc_rot += 1
                for k in range(KC):
                    nc.tensor.matmul(
                        sc[0:cap, j, :], lhsT=xt_sb[:, k, o:o + cap],
                        rhs=w[:, KC * 128 + 8 * k:
                              KC * 128 + 8 * k + 8],
                        start=(k == 0), stop=False)
                nc.tensor.matmul(
                    sc[0:cap, j, :], lhsT=ones_sb[:, o:o + cap],
                    rhs=b8_sb[:, 8 * r:8 * r + 8],
                    start=False, stop=True)
                nc.vector.tensor_copy(out=stg[0:cap, r, :],
                                      in_=sc[0:cap, j, :])
                # first-layer chain: contiguous per rank in its own psum
                # bank (an intervening start=True in a shared bank would
                # reset the open accumulation)
                p1 = p1p.tile([128, cap], F32, tag="p1", name=f"p1_{r}",
                              padded_shape=[128, 128])
                for k in range(KC):
                    nc.tensor.matmul(
                        p1[:], lhsT=w[:, ts(k, 128)],
                        rhs=xt_sb[:, k, o:o + cap], start=(k == 0),
                        stop=(k == KC - 1))
                nc.scalar.activation(out=hf[:, ob:ob + cap], in_=p1[:],
                                     func=F_RELU, bias=b1_sb[:, r:r + 1])

            # --- batched expert selection for this chunk of ranks
            nch = hi - lo
            csl = slice(lo, hi)
            shp = [128, nch]

            def sl(j):
                return stg[:, csl, j]

            b0 = pool.tile(shp, BF16, tag="b0", name=f"b0_{lo}")
            b1 = pool.tile(shp, BF16, tag="b1", name=f"b1_{lo}")
            b2 = pool.tile(shp, BF16, tag="b2", name=f"b2_{lo}")
            s9 = pool.tile(shp, BF16, tag="s9", name=f"s9_{lo}")
            s10 = pool.tile(shp, BF16, tag="s10", name=f"s10_{lo}")
            c0 = pool.tile(shp, BF16, tag="c0", name=f"c0_{lo}")
            c1 = pool.tile(shp, BF16, tag="c1", name=f"c1_{lo}")
            ee = pool.tile(shp, BF16, tag="ee", name=f"ee_{lo}")
            tm = pool.tile(shp, BF16, tag="tm", name=f"tm_{lo}")

            ge = AluOpType.is_ge

            def asel(out_t, b, hi_ap, lo_ap, tmp):
                # out = b ? hi : lo  (b is exactly 0.0/1.0)
                nc.vector.tensor_tensor(out=tmp[:], in0=hi_ap, in1=lo_ap,
                                        op=AluOpType.subtract)
                nc.vector.tensor_tensor(out=tmp[:], in0=b[:], in1=tmp[:],
                                        op=AluOpType.mult)
                nc.vector.tensor_tensor(out=out_t[:], in0=tmp[:], in1=lo_ap,
                                        op=AluOpType.add)

            nc.vector.tensor_scalar(out=b0[:], in0=sl(0), scalar1=0.0,
                                    scalar2=None, op0=ge)
            asel(s9, b0, sl(2), sl(1), tm)
            nc.vector.tensor_scalar(out=b1[:], in0=s9[:], scalar1=0.0,
                                    scalar2=None, op0=ge)
            asel(c0, b0, sl(5), sl(3), tm)
            asel(c1, b0, sl(6), sl(4), tm)
            asel(s10, b1, c1[:], c0[:], tm)
            nc.vector.tensor_scalar(out=b2[:], in0=s10[:], scalar1=0.0,
                                    scalar2=None, op0=ge)
            nc.vector.scalar_tensor_tensor(out=ee[:], in0=b0[:], scalar=2.0,
                                           in1=b1[:], op0=AluOpType.mult,
                                           op1=AluOpType.add)
            nc.vector.scalar_tensor_tensor(out=ee[:], in0=ee[:], scalar=2.0,
                                           in1=b2[:], op0=AluOpType.mult,
                                           op1=AluOpType.add)
            nc.vector.tensor_tensor(
                out=ao[:, csl, :], in0=i8[:, csl, :],
                in1=ee[:, :, None].to_broadcast([128, nch, 8]),
                op=AluOpType.is_equal)
            # margins: min(s8^2, s9^2, s10^2)
            nc.vector.tensor_tensor(out=tm[:], in0=sl(0), in1=sl(0),
                                    op=AluOpType.mult)
            nc.vector.tensor_tensor(out=s9[:], in0=s9[:], in1=s9[:],
                                    op=AluOpType.mult)
            nc.vector.tensor_tensor(out=tm[:], in0=tm[:], in1=s9[:],
                                    op=AluOpType.min)
            nc.vector.tensor_tensor(out=s10[:], in0=s10[:], in1=s10[:],
                                    op=AluOpType.mult)
            nc.vector.tensor_tensor(out=mgt[:, csl], in0=tm[:], in1=s10[:],
                                    op=AluOpType.min)

            # --- transpose anti-one-hots for the chunk's 4-rank clusters
            for cl in range(lo // 4, (hi + 3) // 4):
                tr = trp.tile([32, 128], BF16, tag="tr", name=f"tr{cl}")
                nc.tensor.transpose(
                    tr[:], in_=ao[:, 4 * cl:4 * cl + 4, :], identity=id_sb[:])
                at = apool.tile([32, 128], BF16, tag="at", name=f"at{cl}")
                nc.vector.tensor_copy(out=at[:], in_=tr[:])
                at_tiles[cl] = at

            # --- finalize blocks whose ranks are all streamed
            for t in sorted(hf_tiles):
                if block_last[t] >= hi:
                    continue
                hf = hf_tiles.pop(t)
                mo = mop.tile([128, 128], F32, tag="mo", name=f"mo_{t}")
                for r, ob, cap in blocks[t]:
                    nc.tensor.matmul(
                        mo[:, ob:ob + cap], lhsT=m32_sb[:, r % 4, :],
                        rhs=at_tiles[r // 4][:, 0:cap], start=True,
                        stop=True)
                hm = hpool.tile([128, 128], BF16, tag="hm", name=f"hm{t}")
                nc.vector.tensor_tensor(out=hm[:], in0=hf[:], in1=mo[:],
                                        op=AluOpType.mult)
                if DEBUG_L2:
                    dh = pool.tile([128, 128], F32, tag="dh", name=f"dh{t}",
                                   bufs=2)
                    nc.vector.tensor_copy(out=dh[:], in_=hm[:])
                    nc.sync.dma_start(
                        out=dhf[:, 128 * t:128 * t + 128], in_=dh[:])

                po = pop.tile([128, KC, 128], F32, tag="po", name=f"po{t}")
                brs = [r for r, _, _ in blocks[t]]
                assert brs == list(range(brs[0], brs[0] + len(brs)))
                w2b = w2pool.tile([128, len(brs), W2_W], BF16, tag="w2",
                                  name=f"w2b{t}")
                nc.sync.dma_start(
                    out=w2b[:],
                    in_=wsl2[brs[0]:brs[-1] + 1].rearrange("g p w -> p g w"))
                for i, (r, ob, cap) in enumerate(blocks[t]):
                    for j in range(KC):
                        nc.tensor.matmul(
                            po[:, j, ob:ob + cap],
                            lhsT=w2b[:, i, 128 * j:128 * j + 128],
                            rhs=hm[:, ob:ob + cap], start=True, stop=True)
                wdx = t // 2
                if wdx not in osb_tiles:
                    osb_tiles[wdx] = opool.tile([128, KC, 256], BF16,
                                                tag="osb", name=f"osb{wdx}")
                    osb_cnt[wdx] = 0
                osb = osb_tiles[wdx]
                half = slice(128 * (t % 2), 128 * (t % 2) + 128)
                nc.scalar.activation(out=osb[:, :, half], in_=po[:],
                                     func=F_COPY)
                osb_cnt[wdx] += 1
                need = 1 if (T % 2 == 1 and wdx == T // 2) else 2
                if osb_cnt[wdx] == need:
                    w0 = wdx * 256
                    wid = 128 * need
                    nc.scalar.dma_start(
                        out=outT.rearrange("(j p) s -> p j s",
                                           p=128)[:, :, w0:w0 + wid],
                        in_=osb[:, :, 0:wid])
                    del osb_tiles[wdx]

        nc.scalar.dma_start(out=mg2, in_=mgt[:])
        if DEBUG_L2:
            ds = pool.tile([128, NG, 8], F32)
            nc.vector.tensor_copy(out=ds[:], in_=stg[:])
            nc.sync.dma_start(out=dstg, in_=ds.rearrange("p a b -> p (a b)"))
            da = pool.tile([128, NG, 8], F32)
            nc.vector.tensor_copy(out=da[:], in_=ao[:])
            nc.sync.dma_start(out=dao, in_=da.rearrange("p a b -> p (a b)"))

    nc.compile()
    return nc, plan, T


# ---------------------------------------------------------------- host side
def _host_prep_l1(x, node_weights, node_biases):
    import ml_dtypes
    bf16 = ml_dtypes.bfloat16
    wdv = np.zeros((IN_W, 256), np.float32)
    wdv[:, :255] = node_weights[:255].T
    wdb = np.zeros((1, 256), np.float32)
    wdb[0, :255] = node_biases[:255]
    in_maps = []
    for c in range(N_CORES):
        xs = x[c * B_CORE:(c + 1) * B_CORE]
        in_maps.append({
            "xT": np.ascontiguousarray(xs.T).astype(bf16),
            "wd": wdv.astype(bf16), "wdb": wdb.astype(bf16),
        })
    return in_maps


def _subtree_nodes(G):
    a = 255 + G
    return [a, 2 * a + 1, 2 * a + 2,
            4 * a + 3, 4 * a + 4, 4 * a + 5, 4 * a + 6]


def _host_prep_l2(g8, x, node_weights, node_biases, w1s, b1s, w2s):
    import ml_dtypes
    bf16 = ml_dtypes.bfloat16

    counts = np.zeros((N_CORES, GROUPS_PER_CORE), np.int64)
    core_of = g8 // GROUPS_PER_CORE
    loc = g8 % GROUPS_PER_CORE
    for c in range(N_CORES):
        counts[c] = np.bincount(loc[core_of == c], minlength=GROUPS_PER_CORE)

    orders = [np.argsort(-counts[c], kind="stable") for c in range(N_CORES)]
    ranked = np.stack([counts[c][orders[c]] for c in range(N_CORES)])
    caps = [min(int(m), 128) for m in ranked.max(axis=0)]
    norder, pcaps, plan, T = _plan_blocks(caps)
    SLOTS = 128 * T
    W1_W = KC * 128 + 48

    order_s = np.argsort(g8, kind="stable")  # samples grouped by level-8 node

    in_maps, slot_samples, overflow = [], [], []
    for c in range(N_CORES):
        xgT = np.zeros((IN_W, SLOTS), np.float32)
        wslab1 = np.zeros((GROUPS_PER_CORE, 128, W1_W), np.float32)
        wslab2 = np.zeros((GROUPS_PER_CORE, 128, OUT_W), np.float32)
        b8rv = np.zeros((1, GROUPS_PER_CORE * 8), np.float32)
        b1cv = np.zeros((128, GROUPS_PER_CORE), np.float32)
        ss = np.full(SLOTS, -1, np.int64)

        for r in range(len(norder)):
            gid = int(orders[c][norder[r]])
            G = 32 * c + gid
            cnt = int(counts[c][gid])
            t, ob = plan[r]
            base = 128 * t + ob
            if cnt:
                lo = np.searchsorted(g8[order_s], G)
                samples = order_s[lo:lo + cnt]
                if cnt > pcaps[r]:  # overflow -> host recompute (never in
                    overflow.extend(samples[pcaps[r]:])  # practice)
                    samples = samples[:pcaps[r]]
                    cnt = pcaps[r]
                ss[base:base + cnt] = samples
                xgT[:, base:base + cnt] = x[samples].T

            # w1 stack: [x-dim chunks k of 128][128 lanes (16 per expert)]
            w1f = (w1s[8 * G:8 * G + 8]           # [8, 768, 16]
                   .transpose(1, 0, 2).reshape(IN_W, 128)
                   .reshape(KC, 128, 128))
            wslab1[r, :, :KC * 128] = w1f.transpose(1, 0, 2).reshape(128, -1)
            # w2 stack: [128 lanes, 768]
            wslab2[r] = w2s[8 * G:8 * G + 8].reshape(128, OUT_W)
            # subtree node rows: [x-dim chunks k][8 cols (7 nodes + pad)]
            nodes = _subtree_nodes(G)
            w8 = np.zeros((IN_W, 8), np.float32)
            w8[:, :7] = node_weights[nodes].T
            wslab1[r, :, KC * 128:] = \
                w8.reshape(KC, 128, 8).transpose(1, 0, 2).reshape(128, 48)
            b1cv[:, r] = b1s[8 * G:8 * G + 8].reshape(128)
            b8rv[0, 8 * r:8 * r + 7] = node_biases[nodes]

        m32bv = np.zeros((32, 4 * 128), np.float32)
        for q in range(32):
            pos, j = q // 8, q % 8
            m32bv[q, 128 * pos + 16 * j:128 * pos + 16 * j + 16] = 1.0
        ident = np.eye(128, dtype=np.float32)

        in_maps.append({
            "xgT": xgT.astype(bf16),
            "wsl1": wslab1.astype(bf16),
            "wsl2": wslab2.astype(bf16),
            "b8r": b8rv.astype(bf16), "b1c": b1cv.astype(bf16),
            "m32b": m32bv.astype(bf16),
            "ident": ident.astype(bf16),
        })
        slot_samples.append(ss)
    return in_maps, slot_samples, caps, T, overflow


def _host_reroute_rows(flagged, x, node_weights, node_biases, w1s, b1s, w2s):
    """Reference-faithful recompute of routing + MLP for flagged samples."""
    if not len(flagged):
        return np.zeros((0, OUT_W), np.float32), np.zeros(0, np.int64)
    xs = x[flagged]
    cur = np.zeros(len(flagged), np.int64)
    for _ in range(DEPTH):
        sc = (np.einsum("bi,bi->b", xs.astype(np.float64),
                        node_weights[cur].astype(np.float64))
              + node_biases[cur].astype(np.float64))
        cur = 2 * cur + 1 + (sc >= 0)
    leaves = cur - N_NODES
    h = np.einsum("bi,bil->bl", xs, w1s[leaves]) + b1s[leaves]
    h = np.maximum(h, 0.0)
    rows = np.einsum("bl,blo->bo", h, w2s[leaves]).astype(np.float32)
    return rows, leaves


# ---------------------------------------------------------------- entry
def kernel(x, node_weights, node_biases, w1s, b1s, w2s):
    x = np.ascontiguousarray(np.asarray(x, np.float32))
    node_weights = np.ascontiguousarray(np.asarray(node_weights, np.float32))
    node_biases = np.ascontiguousarray(np.asarray(node_biases, np.float32))
    w1s = np.asarray(w1s, np.float32)
    b1s = np.asarray(b1s, np.float32)
    w2s = np.asarray(w2s, np.float32)

    # launch 1: dense routing levels 0..7
    nc1 = _build_l1_nc()
    in1 = _host_prep_l1(x, node_weights, node_biases)
    res1 = run_bass_kernel_spmd(nc1, in1, core_ids=list(range(N_CORES)))
    g8_l, mg_l = [], []
    for c in range(N_CORES):
        go = np.asarray(res1.results[c]["gout"])
        g8_l.append(go[:, 0:8].T.reshape(-1))
        mg_l.append(go[:, 8:16].T.reshape(-1).view(np.float32))
    g8 = np.concatenate(g8_l).astype(np.int64)
    mg1 = np.concatenate(mg_l)

    # launch 2: fused subtree routing + expert MLP
    in2, slot_samples, caps, T, overflow = _host_prep_l2(
        g8, x, node_weights, node_biases, w1s, b1s, w2s)
    global LAST_CAPS
    LAST_CAPS = caps
    nc2, plan, T2 = _build_l2_nc(caps)
    assert T2 == T
    res2 = run_bass_kernel_spmd(nc2, in2, core_ids=list(range(N_CORES)))

    out = np.zeros((BATCH, OUT_W), np.float32)
    mg2 = np.zeros(BATCH, np.float32)
    for c in range(N_CORES):
        ss = slot_samples[c]
        valid = ss >= 0
        o = np.asarray(res2.results[c]["outT"], dtype=np.float32)
        out[ss[valid]] = o[:, valid].T
        m = np.asarray(res2.results[c]["mg2"], dtype=np.float32)
        slotmg = np.zeros(len(ss), np.float32)
        _, pcaps2, plan2, _ = _plan_blocks(caps)
        for r, (t, ob) in plan2.items():
            base = 128 * t + ob
            slotmg[base:base + pcaps2[r]] = m[0:pcaps2[r], r]
        mg2[ss[valid]] = slotmg[valid]

    flag = (mg1 < MARGIN1 ** 2) | (mg2 < MARGIN2 ** 2)
    flag[np.asarray(overflow, np.int64)] = True
    flagged = np.nonzero(flag)[0]
    rows, _ = _host_reroute_rows(flagged, x, node_weights, node_biases,
                                 w1s, b1s, w2s)
    out[flagged] = rows
    return out

